# revision 13
# baseline (speedup 1.0000x reference)
"""Trainium2 Bass kernel for causal multi-head attention.

Problem: B=2, S=2048, D=1024, H=16 heads (hd=64), fp32 in/out.
  qkv = x @ Wqkv + bqkv ; per-head causal softmax attention ; out = ctx @ Wo + bo

Sharding (8 NeuronCores): tensor-parallel over heads — 2 heads per core.
Each core computes q/k/v projections for its 2 heads (both batches), causal
attention, and its ctx^T slice [128 feat, B*S]. The ctx^T exchange uses four
AllToAll collectives (one per batch-half, 256KB in / 256KB out per core) so
each core receives ONLY the 1024-feature columns it needs for its 128 output
rows per (batch, half) — 8x less wire traffic than an AllGather. Each core
then computes the output projection for its 4x128 rows with the full Wo.
Host reassembles the row slices.

Schedule: qkv chunks interleave with attention windows; batch0 windows run
ascending, batch1 descending so the final window is the cheap 4-tile one
(small pipeline drain); output projections are placed to overlap the
collectives they depend on.

Numerics: bf16 matmul operands, fp32 PSUM accumulation. Softmax uses
exp without max-subtraction (scores are ~N(0,1) after the folded 1/sqrt(hd)
scale). The softmax denominator comes free as a ones-column appended to v in
the attn@v matmul. ctx matmuls skip causally-zero column ranges of the
diagonal k-tiles.
"""

import numpy as np
import ml_dtypes

B, S, D, H, NC = 2, 2048, 1024, 16, 8
HD = D // H            # 64
HPC = H // NC          # 2 heads per core
BS = B * S             # 4096
KC = D // 128          # 8 contraction chunks
SC = BS // 512         # 8 s-chunks for qkv projection
NQT = S // 512         # 4 q-windows (512) per batch
NKT = S // 128         # 16 k-tiles (128) per batch

BF16 = ml_dtypes.bfloat16

_CACHE = {}


def _build_program():
    import concourse.bass as bass
    import concourse.mybir as mybir
    from concourse import bacc
    from concourse.tile import TileContext

    dt = mybir.dt
    f32, bf16 = dt.float32, dt.bfloat16
    ALU = mybir.AluOpType
    ACTF = mybir.ActivationFunctionType

    nc = bacc.Bacc("TRN2", target_bir_lowering=False, debug=False, num_devices=NC)

    xT = nc.dram_tensor("xT", [D, BS], bf16, kind="ExternalInput")
    wqk = nc.dram_tensor("wqk", [D, 256], bf16, kind="ExternalInput")
    wv = nc.dram_tensor("wv", [D, 128], bf16, kind="ExternalInput")
    wo = nc.dram_tensor("wo", [D, D], bf16, kind="ExternalInput")
    bqk = nc.dram_tensor("bqk", [128, 2], f32, kind="ExternalInput")
    bv = nc.dram_tensor("bv", [128, 128], bf16, kind="ExternalInput")
    bo = nc.dram_tensor("bo", [128, D], f32, kind="ExternalInput")
    mask = nc.dram_tensor("mask", [128, 896], bf16, kind="ExternalInput")
    out = nc.dram_tensor("out", [2 * B * 128, D], f32, kind="ExternalOutput")

    # AllToAll buffers: one per (batch, window). Block k of a2a_in goes to
    # core k (its 64-col strip of the window); block k of a2a_out arrives
    # from core k (its 128 features for OUR strip).
    NW = B * NQT
    a2a_in = [nc.dram_tensor(f"a2ain{w}", [NC, 128, 64], bf16)
              for w in range(NW)]
    a2a_out = [nc.dram_tensor(f"a2aout{w}", [NC, 128, 64], bf16)
               for w in range(NW)]

    with TileContext(nc) as tc:
        with (
            tc.tile_pool(name="const", bufs=1) as cpool,
            tc.tile_pool(name="big", bufs=1) as bigpool,
            tc.tile_pool(name="xstream", bufs=2) as xpool,
            tc.tile_pool(name="exp", bufs=1) as epool,
            tc.tile_pool(name="small", bufs=3) as spool,
            tc.tile_pool(name="outp", bufs=2) as opool,
            tc.tile_pool(name="psA", bufs=2, space="PSUM") as psA,   # 2x [128,1536]
            tc.tile_pool(name="psB", bufs=2, space="PSUM") as psB,   # 2x [128,512]
        ):
            # ---- constants / weights to SBUF. Critical-path tensors (wqk,
            # wv, xt chunks) go on the sync DGE queue; the rest ride the
            # scalar queue's DGE in parallel so xt0 isn't stuck behind wo.
            wqk_sb = cpool.tile([128, KC, 256], bf16, tag="wqk")
            nc.sync.dma_start(wqk_sb[:], wqk.rearrange("(ko p) m -> p ko m", p=128))
            bqk_sb = cpool.tile([128, 2], f32, tag="bqk")
            nc.sync.dma_start(bqk_sb[:], bqk[:])
            wv_sb = cpool.tile([128, KC, 128], bf16, tag="wv")
            nc.sync.dma_start(wv_sb[:], wv.rearrange("(ko p) m -> p ko m", p=128))
            bv_sb = cpool.tile([128, 128], bf16, tag="bv")
            nc.scalar.dma_start(bv_sb[:], bv[:])
            mask_sb = cpool.tile([128, 896], bf16, tag="mask")
            nc.scalar.dma_start(mask_sb[:], mask[:])
            wo_sb = cpool.tile([128, KC, D], bf16, tag="wo")
            nc.scalar.dma_start(wo_sb[:], wo.rearrange("(ko p) m -> p ko m", p=128))
            bo_sb = cpool.tile([128, D], f32, tag="bo")
            nc.scalar.dma_start(bo_sb[:], bo[:])

            # ---- persistent activations ----
            qT_sb = bigpool.tile([128, BS], bf16, tag="qT")   # [2*64 feat, B*S]
            kT_sb = bigpool.tile([128, BS], bf16, tag="kT")
            # v natural layout + ones cols: per 128-row chunk:
            #   [v_h0(0:64) | ones(64) | v_h1(65:129) | ones(129)]
            v_sb = bigpool.tile([128, BS // 128, 130], bf16, tag="v")
            # ctx^T in 128-col tiles: [feat part, tile, col]
            ctxT_sb = bigpool.tile([128, BS // 128, 128], bf16, tag="ctxT")

            nc.vector.memset(v_sb[:, :, 64:65], 1.0)
            nc.vector.memset(v_sb[:, :, 129:130], 1.0)

            xT_r = xT.rearrange("(ko p) s -> p ko s", p=128)

            # Packed diagonal-band layout (bank-aligned PSUM offsets): the 4
            # diagonal k-tiles of a window only reach cols >= 128*o, packed
            # at offsets 0/512/1024/1280 with sizes 512/384/256/128.
            DOFF = [0, 512, 1024, 1280]
            DLEN = [512, 384, 256, 128]
            DTOT = 1408

            # ---- qkv projection for one 512-position chunk ----
            def emit_qkv_chunk(sc):
                xt = xpool.tile([128, KC, 512], bf16, tag="xt")
                if sc == 0:
                    # split the first chunk's load so the first matmuls can
                    # start after half the data
                    nc.sync.dma_start(xt[:, 0:KC // 2, :],
                                      xT_r[:, 0:KC // 2, 0:512])
                    nc.sync.dma_start(xt[:, KC // 2:KC, :],
                                      xT_r[:, KC // 2:KC, 0:512])
                else:
                    nc.sync.dma_start(xt[:], xT_r[:, :, sc * 512:(sc + 1) * 512])

                ps = psA.tile([128, 1536], f32, tag="psA", name="ps_qk")
                ps_q = ps[:, 0:512]
                ps_k = ps[:, 512:1024]
                for kk in range(KC):
                    nc.tensor.matmul(ps_q, lhsT=wqk_sb[:, kk, 0:128],
                                     rhs=xt[:, kk, :],
                                     start=(kk == 0), stop=(kk == KC - 1))
                for kk in range(KC):
                    nc.tensor.matmul(ps_k, lhsT=wqk_sb[:, kk, 128:256],
                                     rhs=xt[:, kk, :],
                                     start=(kk == 0), stop=(kk == KC - 1))
                qs = slice(sc * 512, (sc + 1) * 512)
                nc.vector.tensor_scalar_add(qT_sb[:, qs], ps_q, bqk_sb[:, 0:1])
                nc.vector.tensor_scalar_add(kT_sb[:, qs], ps_k, bqk_sb[:, 1:2])

                for s4 in range(4):
                    vps = ps[:, 1024 + s4 * 128:1024 + (s4 + 1) * 128]
                    for kk in range(KC):
                        nc.tensor.matmul(
                            vps,
                            lhsT=xt[:, kk, s4 * 128:(s4 + 1) * 128],
                            rhs=wv_sb[:, kk, :],
                            start=(kk == 0), stop=(kk == KC - 1))
                    sidx = sc * 4 + s4
                    nc.vector.tensor_tensor(v_sb[:, sidx, 0:64], vps[:, 0:64],
                                            bv_sb[:, 0:64], ALU.add)
                    nc.vector.tensor_tensor(v_sb[:, sidx, 65:129], vps[:, 64:128],
                                            bv_sb[:, 64:128], ALU.add)

            # ---- scores + exp + mask for window (b, j); returns exp tiles.
            # exp tile layout: 4j full 512-col k-tile slots, then the packed
            # 1408-col diagonal band. ----
            def emit_scores(b, j):
                eoff = 4 * j * 512
                exp_js = []
                for hl in range(HPC):
                    exp_js.append(epool.tile([128, eoff + DTOT], bf16,
                                             tag=f"expj{j}h{hl}",
                                             name="exp_j"))
                for hl in range(HPC):
                    hp = slice(64 * hl, 64 * hl + 64)
                    qwin = qT_sb[hp,
                                 b * S + j * 512: b * S + (j + 1) * 512]
                    # full k-tiles in pieces of 3
                    tt = 0
                    while tt < 4 * j:
                        npc = min(3, 4 * j - tt)
                        ps = psA.tile([128, 1536], f32, tag="psA",
                                      name="ps_sc")[:, :npc * 512]
                        for i in range(npc):
                            kt = b * S + (tt + i) * 128
                            nc.tensor.matmul(
                                ps[:, i * 512:(i + 1) * 512],
                                lhsT=kT_sb[hp, kt:kt + 128],
                                rhs=qwin,
                                start=True, stop=True)
                        nc.scalar.activation(
                            exp_js[hl][:, tt * 512:(tt + npc) * 512],
                            ps, ACTF.Exp)
                        tt += npc
                    # packed diagonal band: two pieces (o=0,1 and o=2,3)
                    ps = psA.tile([128, 1536], f32, tag="psA", name="ps_sd")
                    for o in range(4):
                        kt = b * S + (4 * j + o) * 128
                        nc.tensor.matmul(
                            ps[:, DOFF[o]:DOFF[o] + DLEN[o]],
                            lhsT=kT_sb[hp, kt:kt + 128],
                            rhs=qwin[:, 128 * o:512],
                            start=True, stop=True)
                    # one op over the band; the [896:1024] gap holds stale
                    # PSUM (always previously written) and is never read
                    nc.scalar.activation(
                        exp_js[hl][:, eoff:eoff + DTOT],
                        ps[:, 0:DTOT], ACTF.Exp)
                    # triangle mask on the first 128 cols of each diag block
                    for o in range(4):
                        blk = slice(eoff + DOFF[o], eoff + DOFF[o] + 128)
                        nc.vector.tensor_tensor(exp_js[hl][:, blk],
                                                exp_js[hl][:, blk],
                                                mask_sb[:, 384:512],
                                                ALU.mult)
                return exp_js

            # ---- attn @ v for window (b, j), one head; causally trimmed ----
            def emit_ctx(b, hl, j, exp_j):
                hp = slice(64 * hl, 64 * hl + 64)
                eoff = 4 * j * 512
                ps_c = psB.tile([128, 512], f32, tag="psB", name="ps_c")
                # full k-tiles below the diagonal band
                for tt in range(4 * j):
                    nc.tensor.matmul(
                        ps_c[:65, :],
                        lhsT=v_sb[:, b * NKT + tt, 65 * hl: 65 * hl + 65],
                        rhs=exp_j[:, tt * 512:(tt + 1) * 512],
                        start=(tt == 0), stop=False)
                # diagonal band: tile 4j+o only reaches cols >= 128*o
                for o in range(4):
                    tt = 4 * j + o
                    nc.tensor.matmul(
                        ps_c[:65, 128 * o:512],
                        lhsT=v_sb[:, b * NKT + tt, 65 * hl: 65 * hl + 65],
                        rhs=exp_j[:, eoff + DOFF[o]:eoff + DOFF[o] + DLEN[o]],
                        start=(j == 0 and o == 0), stop=(o == 3),
                        skip_group_check=True)
                tb = b * NKT + 4 * j
                nc.vector.tensor_copy(ctxT_sb[hp, tb:tb + 4, :], ps_c[0:64, :])
                den = spool.tile([1, 512], f32, tag="den")
                nc.vector.tensor_copy(den[:], ps_c[64:65, :])
                recip = spool.tile([1, 512], f32, tag="recip")
                nc.vector.reciprocal_approx_fast(out=recip[:], in_=den[:])
                bcast = spool.tile([128, 512], f32, tag="bcast")
                nc.gpsimd.partition_broadcast(bcast[:], recip[:])
                nc.vector.tensor_tensor(ctxT_sb[hp, tb:tb + 4, :],
                                        ctxT_sb[hp, tb:tb + 4, :],
                                        bcast[hp, :], ALU.mult)

            # ---- AllToAll for window w = b*NQT + j ----
            def emit_a2a(b, j):
                w = b * NQT + j
                tb = b * NKT + 4 * j
                # block k = ctxT cols [64k, 64k+64) of the window
                nc.sync.dma_start(
                    a2a_in[w].rearrange("(t two) p c -> p t two c", two=2),
                    ctxT_sb[:, tb:tb + 4, :].rearrange(
                        "p t (two c) -> p t two c", two=2))
                nc.gpsimd.collective_compute(
                    "AllToAll",
                    mybir.AluOpType.bypass,
                    replica_groups=[list(range(NC))],
                    ins=[a2a_in[w][:]],
                    outs=[a2a_out[w][:]],
                )

            # ---- output projection for group g = 2*b + half (128 rows:
            # 64 from window 2*half, 64 from window 2*half+1) ----
            def emit_proj(g):
                b, half = g // 2, g % 2
                asb = opool.tile([128, NC, 128], bf16, tag="asb", name="asb")
                for sub in range(2):
                    w = b * NQT + 2 * half + sub
                    nc.sync.dma_start(
                        asb[:, :, sub * 64:(sub + 1) * 64],
                        a2a_out[w].rearrange("k p c -> p k c"))
                ot = opool.tile([128, D], f32, tag="ot")
                for ncol in range(D // 512):
                    ps_o = psB.tile([128, 512], f32, tag="psB", name="ps_o")
                    for k in range(NC):
                        nc.tensor.matmul(
                            ps_o,
                            lhsT=asb[:, k, :],
                            rhs=wo_sb[:, k, ncol * 512:(ncol + 1) * 512],
                            start=(k == 0), stop=(k == NC - 1))
                    cs = slice(ncol * 512, (ncol + 1) * 512)
                    nc.vector.tensor_tensor(ot[:, cs], ps_o,
                                            bo_sb[:, cs], ALU.add)
                    nc.sync.dma_start(out[g * 128:(g + 1) * 128, cs],
                                      ot[:, cs])

            # ---- schedule ----
            scope1 = nc.named_scope("main"); scope1.__enter__()
            exp_w = {}

            emit_qkv_chunk(0)
            emit_qkv_chunk(1)
            exp_w[0] = emit_scores(0, 0)
            emit_qkv_chunk(2)
            exp_w[1] = emit_scores(0, 1)
            for hl in range(HPC):
                emit_ctx(0, hl, 0, exp_w[0][hl])
            emit_a2a(0, 0)
            emit_qkv_chunk(3)
            exp_w[2] = emit_scores(0, 2)
            for hl in range(HPC):
                emit_ctx(0, hl, 1, exp_w[1][hl])
            emit_a2a(0, 1)
            emit_qkv_chunk(4)
            exp_w[3] = emit_scores(0, 3)
            for hl in range(HPC):
                emit_ctx(0, hl, 2, exp_w[2][hl])
            emit_a2a(0, 2)
            emit_qkv_chunk(5)
            emit_qkv_chunk(6)
            for hl in range(HPC):
                emit_ctx(0, hl, 3, exp_w[3][hl])
            emit_a2a(0, 3)
            emit_qkv_chunk(7)
            # batch 1, descending windows: the last window is the cheap one
            exp_w[3] = emit_scores(1, 3)
            emit_proj(0)
            exp_w[2] = emit_scores(1, 2)
            for hl in range(HPC):
                emit_ctx(1, hl, 3, exp_w[3][hl])
            emit_a2a(1, 3)
            exp_w[1] = emit_scores(1, 1)
            for hl in range(HPC):
                emit_ctx(1, hl, 2, exp_w[2][hl])
            emit_a2a(1, 2)
            emit_proj(1)
            exp_w[0] = emit_scores(1, 0)
            for hl in range(HPC):
                emit_ctx(1, hl, 1, exp_w[1][hl])
            emit_a2a(1, 1)
            for hl in range(HPC):
                emit_ctx(1, hl, 0, exp_w[0][hl])
            emit_a2a(1, 0)
            # held-back projections overlap the final AllToAlls
            emit_proj(3)
            emit_proj(2)

            scope1.__exit__(None, None, None)

    nc.compile()
    return nc


def _prep_inputs(x, Wqkv, bqkv, Wo, bo):
    x = np.asarray(x, dtype=np.float32)
    Wqkv = np.asarray(Wqkv, dtype=np.float32)
    bqkv = np.asarray(bqkv, dtype=np.float32)
    Wo = np.asarray(Wo, dtype=np.float32)
    bo = np.asarray(bo, dtype=np.float32)

    xT = np.ascontiguousarray(x.reshape(BS, D).T).astype(BF16)
    wo_b = Wo.astype(BF16)
    bo_t = np.tile(bo.astype(np.float32), (128, 1))

    kp = np.arange(128)[:, None]
    u = np.arange(896)[None, :]
    mask = (u >= 384 + kp).astype(BF16)

    scale = np.float32(1.0 / np.sqrt(HD))

    # Wqkv columns per head h: q = 192h..+64, k = +64, v = +128
    W3 = Wqkv.reshape(D, H, 3, HD)
    b3 = bqkv.reshape(H, 3, HD)

    in_maps = []
    for c in range(NC):
        hs = [HPC * c + i for i in range(HPC)]
        wq = np.concatenate([W3[:, h, 0, :] for h in hs], axis=1) * scale
        wk = np.concatenate([W3[:, h, 1, :] for h in hs], axis=1)
        wv_ = np.concatenate([W3[:, h, 2, :] for h in hs], axis=1)
        bq = np.concatenate([b3[h, 0, :] for h in hs]) * scale
        bk = np.concatenate([b3[h, 1, :] for h in hs])
        bv_ = np.concatenate([b3[h, 2, :] for h in hs])
        in_maps.append({
            "xT": xT,
            "wqk": np.ascontiguousarray(
                np.concatenate([wq, wk], axis=1)).astype(BF16),
            "wv": np.ascontiguousarray(wv_).astype(BF16),
            "wo": wo_b,
            "bqk": np.ascontiguousarray(
                np.stack([bq, bk], axis=1)).astype(np.float32),
            "bv": np.tile(bv_.astype(BF16), (128, 1)),
            "bo": bo_t,
            "mask": mask,
        })
    return in_maps


def run(x, Wqkv, bqkv, Wo, bo, trace=False):
    from concourse.bass_utils import run_bass_kernel_spmd

    if "nc" not in _CACHE:
        _CACHE["nc"] = _build_program()
    nc = _CACHE["nc"]
    in_maps = _prep_inputs(x, Wqkv, bqkv, Wo, bo)
    res = run_bass_kernel_spmd(nc, in_maps, list(range(NC)), trace=trace)
    # core c returns [512, D]: 4 groups g=(b, half) of 128 rows; within a
    # group, rows 0:64 are window 2*half strip [64c, 64c+64), rows 64:128
    # are window 2*half+1 strip.
    full = np.empty((B, S, D), dtype=np.float32)
    for c in range(NC):
        r = res.results[c]["out"]
        for g in range(4):
            b, half = g // 2, g % 2
            for sub in range(2):
                lo = half * 1024 + sub * 512 + 64 * c
                full[b, lo:lo + 64, :] = \
                    r[g * 128 + sub * 64:g * 128 + sub * 64 + 64, :]
    return full, res


def kernel(x, Wqkv, bqkv, Wo, bo):
    out, _ = run(x, Wqkv, bqkv, Wo, bo)
    return out


# revision 16
# speedup vs baseline: 1.0871x; 1.0871x over previous
"""Trainium2 Bass kernel for causal multi-head attention.

Problem: B=2, S=2048, D=1024, H=16 heads (hd=64), fp32 in/out.
  qkv = x @ Wqkv + bqkv ; per-head causal softmax attention ; out = ctx @ Wo + bo

Sharding (8 NeuronCores): tensor-parallel over heads — 2 heads per core.
Each core computes q/k/v projections for its 2 heads (both batches), causal
attention, and its ctx^T slice [128 feat, B*S]. The ctx^T exchange uses four
AllToAll collectives (one per batch-half, 256KB in / 256KB out per core) so
each core receives ONLY the 1024-feature columns it needs for its 128 output
rows per (batch, half) — 8x less wire traffic than an AllGather. Each core
then computes the output projection for its 4x128 rows with the full Wo.
Host reassembles the row slices.

Schedule: qkv chunks interleave with attention windows; batch0 windows run
ascending, batch1 descending so the final window is the cheap 4-tile one
(small pipeline drain); output projections are placed to overlap the
collectives they depend on.

Numerics: bf16 matmul operands, fp32 PSUM accumulation. Softmax uses
exp without max-subtraction (scores are ~N(0,1) after the folded 1/sqrt(hd)
scale). The softmax denominator comes free as a ones-column appended to v in
the attn@v matmul. ctx matmuls skip causally-zero column ranges of the
diagonal k-tiles.
"""

import numpy as np
import ml_dtypes

B, S, D, H, NC = 2, 2048, 1024, 16, 8
HD = D // H            # 64
HPC = H // NC          # 2 heads per core
BS = B * S             # 4096
KC = D // 128          # 8 contraction chunks
SC = BS // 512         # 8 s-chunks for qkv projection
NQT = S // 512         # 4 q-windows (512) per batch
NKT = S // 128         # 16 k-tiles (128) per batch

BF16 = ml_dtypes.bfloat16

_CACHE = {}


def _build_program():
    import concourse.bass as bass
    import concourse.mybir as mybir
    from concourse import bacc
    from concourse.tile import TileContext

    dt = mybir.dt
    f32, bf16 = dt.float32, dt.bfloat16
    ALU = mybir.AluOpType
    ACTF = mybir.ActivationFunctionType

    nc = bacc.Bacc("TRN2", target_bir_lowering=False, debug=False, num_devices=NC)

    xT = nc.dram_tensor("xT", [D, BS], bf16, kind="ExternalInput")
    wqk = nc.dram_tensor("wqk", [D, 256], bf16, kind="ExternalInput")
    wv = nc.dram_tensor("wv", [D, 128], bf16, kind="ExternalInput")
    wo = nc.dram_tensor("wo", [D, D], bf16, kind="ExternalInput")
    bqk = nc.dram_tensor("bqk", [128, 2], f32, kind="ExternalInput")
    bv = nc.dram_tensor("bv", [128, 128], bf16, kind="ExternalInput")
    bo = nc.dram_tensor("bo", [128, D], f32, kind="ExternalInput")
    mask = nc.dram_tensor("mask", [128, 896], bf16, kind="ExternalInput")
    out = nc.dram_tensor("out", [2 * B * 128, D], f32, kind="ExternalOutput")

    # AllToAll buffers: one per (batch, window). Block k of a2a_in goes to
    # core k (its 64-col strip of the window); block k of a2a_out arrives
    # from core k (its 128 features for OUR strip).
    NW = B * NQT
    a2a_in = [nc.dram_tensor(f"a2ain{w}", [NC, 128, 64], bf16)
              for w in range(NW)]
    a2a_out = [nc.dram_tensor(f"a2aout{w}", [NC, 128, 64], bf16)
               for w in range(NW)]

    with TileContext(nc) as tc:
        with (
            tc.tile_pool(name="const", bufs=1) as cpool,
            tc.tile_pool(name="big", bufs=1) as bigpool,
            tc.tile_pool(name="xstream", bufs=2) as xpool,
            tc.tile_pool(name="exp", bufs=1) as epool,
            tc.tile_pool(name="small", bufs=3) as spool,
            tc.tile_pool(name="outp", bufs=2) as opool,
            tc.tile_pool(name="psA", bufs=2, space="PSUM") as psA,   # 2x [128,1536]
            tc.tile_pool(name="psB", bufs=2, space="PSUM") as psB,   # 2x [128,512]
        ):
            # ---- constants / weights to SBUF. Critical-path tensors (wqk,
            # wv, xt chunks) go on the sync DGE queue; the rest ride the
            # scalar queue's DGE in parallel so xt0 isn't stuck behind wo.
            wqk_sb = cpool.tile([128, KC, 256], bf16, tag="wqk")
            nc.sync.dma_start(wqk_sb[:], wqk.rearrange("(ko p) m -> p ko m", p=128))
            bqk_sb = cpool.tile([128, 2], f32, tag="bqk")
            nc.sync.dma_start(bqk_sb[:], bqk[:])
            wv_sb = cpool.tile([128, KC, 128], bf16, tag="wv")
            nc.sync.dma_start(wv_sb[:], wv.rearrange("(ko p) m -> p ko m", p=128))
            bv_sb = cpool.tile([128, 128], bf16, tag="bv")
            nc.sync.dma_start(bv_sb[:], bv[:])
            mask_sb = cpool.tile([128, 896], bf16, tag="mask")
            nc.sync.dma_start(mask_sb[:], mask[:])
            wo_sb = cpool.tile([128, KC, D], bf16, tag="wo")
            bo_sb = cpool.tile([128, D], f32, tag="bo")

            def emit_wo_dma():
                nc.sync.dma_start(wo_sb[:],
                                  wo.rearrange("(ko p) m -> p ko m", p=128))
                nc.sync.dma_start(bo_sb[:], bo[:])

            # ---- persistent activations ----
            qT_sb = bigpool.tile([128, BS], bf16, tag="qT")   # [2*64 feat, B*S]
            kT_sb = bigpool.tile([128, BS], bf16, tag="kT")
            # v natural layout + ones cols: per 128-row chunk:
            #   [v_h0(0:64) | ones(64) | v_h1(65:129) | ones(129)]
            v_sb = bigpool.tile([128, BS // 128, 130], bf16, tag="v")
            # ctx^T in 128-col tiles: [feat part, tile, col]
            ctxT_sb = bigpool.tile([128, BS // 128, 128], bf16, tag="ctxT")

            nc.vector.memset(v_sb[:, :, 64:65], 1.0)
            nc.vector.memset(v_sb[:, :, 129:130], 1.0)

            xT_r = xT.rearrange("(ko p) s -> p ko s", p=128)

            # Packed diagonal-band layout (bank-aligned PSUM offsets): the 4
            # diagonal k-tiles of a window only reach cols >= 128*o, packed
            # at offsets 0/512/1024/1280 with sizes 512/384/256/128.
            DOFF = [0, 512, 1024, 1280]
            DLEN = [512, 384, 256, 128]
            DTOT = 1408

            # ---- qkv projection for one 512-position chunk ----
            def emit_qkv_chunk(sc):
                xt = xpool.tile([128, KC, 512], bf16, tag="xt")
                if sc == 0:
                    # split the first chunk's load so the first matmuls can
                    # start after half the data
                    nc.sync.dma_start(xt[:, 0:KC // 2, :],
                                      xT_r[:, 0:KC // 2, 0:512])
                    nc.sync.dma_start(xt[:, KC // 2:KC, :],
                                      xT_r[:, KC // 2:KC, 0:512])
                else:
                    nc.sync.dma_start(xt[:], xT_r[:, :, sc * 512:(sc + 1) * 512])

                ps = psA.tile([128, 1536], f32, tag="psA", name="ps_qk")
                ps_q = ps[:, 0:512]
                ps_k = ps[:, 512:1024]
                for kk in range(KC):
                    nc.tensor.matmul(ps_q, lhsT=wqk_sb[:, kk, 0:128],
                                     rhs=xt[:, kk, :],
                                     start=(kk == 0), stop=(kk == KC - 1))
                for kk in range(KC):
                    nc.tensor.matmul(ps_k, lhsT=wqk_sb[:, kk, 128:256],
                                     rhs=xt[:, kk, :],
                                     start=(kk == 0), stop=(kk == KC - 1))
                qs = slice(sc * 512, (sc + 1) * 512)
                nc.vector.tensor_scalar_add(qT_sb[:, qs], ps_q, bqk_sb[:, 0:1])
                nc.vector.tensor_scalar_add(kT_sb[:, qs], ps_k, bqk_sb[:, 1:2])

                for s4 in range(4):
                    vps = ps[:, 1024 + s4 * 128:1024 + (s4 + 1) * 128]
                    for kk in range(KC):
                        nc.tensor.matmul(
                            vps,
                            lhsT=xt[:, kk, s4 * 128:(s4 + 1) * 128],
                            rhs=wv_sb[:, kk, :],
                            start=(kk == 0), stop=(kk == KC - 1))
                    sidx = sc * 4 + s4
                    nc.vector.tensor_tensor(v_sb[:, sidx, 0:64], vps[:, 0:64],
                                            bv_sb[:, 0:64], ALU.add)
                    nc.vector.tensor_tensor(v_sb[:, sidx, 65:129], vps[:, 64:128],
                                            bv_sb[:, 64:128], ALU.add)

            # ---- scores + exp + mask for window (b, j); returns exp tiles.
            # exp tile layout: 4j full 512-col k-tile slots, then the packed
            # 1408-col diagonal band. ----
            def emit_scores(b, j):
                eoff = 4 * j * 512
                exp_js = []
                for hl in range(HPC):
                    exp_js.append(epool.tile([128, eoff + DTOT], bf16,
                                             tag=f"expj{j}h{hl}",
                                             name="exp_j"))
                for hl in range(HPC):
                    hp = slice(64 * hl, 64 * hl + 64)
                    qwin = qT_sb[hp,
                                 b * S + j * 512: b * S + (j + 1) * 512]
                    # full k-tiles in pieces of 3
                    tt = 0
                    while tt < 4 * j:
                        npc = min(3, 4 * j - tt)
                        ps = psA.tile([128, 1536], f32, tag="psA",
                                      name="ps_sc")[:, :npc * 512]
                        for i in range(npc):
                            kt = b * S + (tt + i) * 128
                            nc.tensor.matmul(
                                ps[:, i * 512:(i + 1) * 512],
                                lhsT=kT_sb[hp, kt:kt + 128],
                                rhs=qwin,
                                start=True, stop=True)
                        nc.scalar.activation(
                            exp_js[hl][:, tt * 512:(tt + npc) * 512],
                            ps, ACTF.Exp)
                        tt += npc
                    # packed diagonal band: two pieces (o=0,1 and o=2,3)
                    ps = psA.tile([128, 1536], f32, tag="psA", name="ps_sd")
                    for o in range(4):
                        kt = b * S + (4 * j + o) * 128
                        nc.tensor.matmul(
                            ps[:, DOFF[o]:DOFF[o] + DLEN[o]],
                            lhsT=kT_sb[hp, kt:kt + 128],
                            rhs=qwin[:, 128 * o:512],
                            start=True, stop=True)
                    # one op over the band; the [896:1024] gap holds stale
                    # PSUM (always previously written) and is never read
                    nc.scalar.activation(
                        exp_js[hl][:, eoff:eoff + DTOT],
                        ps[:, 0:DTOT], ACTF.Exp)
                    # triangle mask on the first 128 cols of each diag block
                    for o in range(4):
                        blk = slice(eoff + DOFF[o], eoff + DOFF[o] + 128)
                        nc.vector.tensor_tensor(exp_js[hl][:, blk],
                                                exp_js[hl][:, blk],
                                                mask_sb[:, 384:512],
                                                ALU.mult)
                return exp_js

            # ---- attn @ v for window (b, j), one head; causally trimmed ----
            def emit_ctx(b, hl, j, exp_j):
                hp = slice(64 * hl, 64 * hl + 64)
                eoff = 4 * j * 512
                ps_c = psB.tile([128, 512], f32, tag="psB", name="ps_c")
                # full k-tiles below the diagonal band
                for tt in range(4 * j):
                    nc.tensor.matmul(
                        ps_c[:65, :],
                        lhsT=v_sb[:, b * NKT + tt, 65 * hl: 65 * hl + 65],
                        rhs=exp_j[:, tt * 512:(tt + 1) * 512],
                        start=(tt == 0), stop=False)
                # diagonal band: tile 4j+o only reaches cols >= 128*o
                for o in range(4):
                    tt = 4 * j + o
                    nc.tensor.matmul(
                        ps_c[:65, 128 * o:512],
                        lhsT=v_sb[:, b * NKT + tt, 65 * hl: 65 * hl + 65],
                        rhs=exp_j[:, eoff + DOFF[o]:eoff + DOFF[o] + DLEN[o]],
                        start=(j == 0 and o == 0), stop=(o == 3),
                        skip_group_check=True)
                tb = b * NKT + 4 * j
                nc.vector.tensor_copy(ctxT_sb[hp, tb:tb + 4, :], ps_c[0:64, :])
                den = spool.tile([1, 512], f32, tag="den")
                nc.vector.tensor_copy(den[:], ps_c[64:65, :])
                recip = spool.tile([1, 512], f32, tag="recip")
                nc.vector.reciprocal_approx_fast(out=recip[:], in_=den[:])
                bcast = spool.tile([128, 512], f32, tag="bcast")
                nc.gpsimd.partition_broadcast(bcast[:], recip[:])
                nc.vector.tensor_tensor(ctxT_sb[hp, tb:tb + 4, :],
                                        ctxT_sb[hp, tb:tb + 4, :],
                                        bcast[hp, :], ALU.mult)

            # ---- AllToAll for window w = b*NQT + j ----
            def emit_a2a(b, j):
                w = b * NQT + j
                tb = b * NKT + 4 * j
                # block k = ctxT cols [64k, 64k+64) of the window
                nc.sync.dma_start(
                    a2a_in[w].rearrange("(t two) p c -> p t two c", two=2),
                    ctxT_sb[:, tb:tb + 4, :].rearrange(
                        "p t (two c) -> p t two c", two=2))
                nc.gpsimd.collective_compute(
                    "AllToAll",
                    mybir.AluOpType.bypass,
                    replica_groups=[list(range(NC))],
                    ins=[a2a_in[w][:]],
                    outs=[a2a_out[w][:]],
                )

            # ---- output projection for group g = 2*b + half (128 rows:
            # 64 from window 2*half, 64 from window 2*half+1) ----
            def emit_proj(g):
                b, half = g // 2, g % 2
                asb = opool.tile([128, NC, 128], bf16, tag="asb", name="asb")
                for sub in range(2):
                    w = b * NQT + 2 * half + sub
                    nc.sync.dma_start(
                        asb[:, :, sub * 64:(sub + 1) * 64],
                        a2a_out[w].rearrange("k p c -> p k c"))
                ot = opool.tile([128, D], f32, tag="ot")
                for ncol in range(D // 512):
                    ps_o = psB.tile([128, 512], f32, tag="psB", name="ps_o")
                    for k in range(NC):
                        nc.tensor.matmul(
                            ps_o,
                            lhsT=asb[:, k, :],
                            rhs=wo_sb[:, k, ncol * 512:(ncol + 1) * 512],
                            start=(k == 0), stop=(k == NC - 1))
                    cs = slice(ncol * 512, (ncol + 1) * 512)
                    nc.vector.tensor_tensor(ot[:, cs], ps_o,
                                            bo_sb[:, cs], ALU.add)
                    nc.sync.dma_start(out[g * 128:(g + 1) * 128, cs],
                                      ot[:, cs])

            # ---- schedule ----
            scope1 = nc.named_scope("main"); scope1.__enter__()
            exp_w = {}

            emit_qkv_chunk(0)
            emit_qkv_chunk(1)
            exp_w[0] = emit_scores(0, 0)
            emit_qkv_chunk(2)
            exp_w[1] = emit_scores(0, 1)
            for hl in range(HPC):
                emit_ctx(0, hl, 0, exp_w[0][hl])
            emit_a2a(0, 0)
            emit_wo_dma()
            emit_qkv_chunk(3)
            exp_w[2] = emit_scores(0, 2)
            for hl in range(HPC):
                emit_ctx(0, hl, 1, exp_w[1][hl])
            emit_a2a(0, 1)
            emit_qkv_chunk(4)
            exp_w[3] = emit_scores(0, 3)
            for hl in range(HPC):
                emit_ctx(0, hl, 2, exp_w[2][hl])
            emit_a2a(0, 2)
            emit_qkv_chunk(5)
            emit_qkv_chunk(6)
            for hl in range(HPC):
                emit_ctx(0, hl, 3, exp_w[3][hl])
            emit_a2a(0, 3)
            emit_qkv_chunk(7)
            # batch 1, descending windows: the last window is the cheap one
            exp_w[3] = emit_scores(1, 3)
            exp_w[2] = emit_scores(1, 2)
            for hl in range(HPC):
                emit_ctx(1, hl, 3, exp_w[3][hl])
            emit_a2a(1, 3)
            exp_w[1] = emit_scores(1, 1)
            for hl in range(HPC):
                emit_ctx(1, hl, 2, exp_w[2][hl])
            emit_a2a(1, 2)
            exp_w[0] = emit_scores(1, 0)
            for hl in range(HPC):
                emit_ctx(1, hl, 1, exp_w[1][hl])
            emit_a2a(1, 1)
            for hl in range(HPC):
                emit_ctx(1, hl, 0, exp_w[0][hl])
            emit_a2a(1, 0)
            # all projections at the end: guaranteed-ready PE work that
            # hides the final collectives regardless of cross-core skew
            emit_proj(0)
            emit_proj(1)
            emit_proj(3)
            emit_proj(2)

            scope1.__exit__(None, None, None)

    nc.compile()
    return nc


def _prep_inputs(x, Wqkv, bqkv, Wo, bo):
    x = np.asarray(x, dtype=np.float32)
    Wqkv = np.asarray(Wqkv, dtype=np.float32)
    bqkv = np.asarray(bqkv, dtype=np.float32)
    Wo = np.asarray(Wo, dtype=np.float32)
    bo = np.asarray(bo, dtype=np.float32)

    xT = np.ascontiguousarray(x.reshape(BS, D).T).astype(BF16)
    wo_b = Wo.astype(BF16)
    bo_t = np.tile(bo.astype(np.float32), (128, 1))

    kp = np.arange(128)[:, None]
    u = np.arange(896)[None, :]
    mask = (u >= 384 + kp).astype(BF16)

    scale = np.float32(1.0 / np.sqrt(HD))

    # Wqkv columns per head h: q = 192h..+64, k = +64, v = +128
    W3 = Wqkv.reshape(D, H, 3, HD)
    b3 = bqkv.reshape(H, 3, HD)

    in_maps = []
    for c in range(NC):
        hs = [HPC * c + i for i in range(HPC)]
        wq = np.concatenate([W3[:, h, 0, :] for h in hs], axis=1) * scale
        wk = np.concatenate([W3[:, h, 1, :] for h in hs], axis=1)
        wv_ = np.concatenate([W3[:, h, 2, :] for h in hs], axis=1)
        bq = np.concatenate([b3[h, 0, :] for h in hs]) * scale
        bk = np.concatenate([b3[h, 1, :] for h in hs])
        bv_ = np.concatenate([b3[h, 2, :] for h in hs])
        in_maps.append({
            "xT": xT,
            "wqk": np.ascontiguousarray(
                np.concatenate([wq, wk], axis=1)).astype(BF16),
            "wv": np.ascontiguousarray(wv_).astype(BF16),
            "wo": wo_b,
            "bqk": np.ascontiguousarray(
                np.stack([bq, bk], axis=1)).astype(np.float32),
            "bv": np.tile(bv_.astype(BF16), (128, 1)),
            "bo": bo_t,
            "mask": mask,
        })
    return in_maps


def run(x, Wqkv, bqkv, Wo, bo, trace=False):
    from concourse.bass_utils import run_bass_kernel_spmd

    if "nc" not in _CACHE:
        _CACHE["nc"] = _build_program()
    nc = _CACHE["nc"]
    in_maps = _prep_inputs(x, Wqkv, bqkv, Wo, bo)
    res = run_bass_kernel_spmd(nc, in_maps, list(range(NC)), trace=trace)
    # core c returns [512, D]: 4 groups g=(b, half) of 128 rows; within a
    # group, rows 0:64 are window 2*half strip [64c, 64c+64), rows 64:128
    # are window 2*half+1 strip.
    full = np.empty((B, S, D), dtype=np.float32)
    for c in range(NC):
        r = res.results[c]["out"]
        for g in range(4):
            b, half = g // 2, g % 2
            for sub in range(2):
                lo = half * 1024 + sub * 512 + 64 * c
                full[b, lo:lo + 64, :] = \
                    r[g * 128 + sub * 64:g * 128 + sub * 64 + 64, :]
    return full, res


def kernel(x, Wqkv, bqkv, Wo, bo):
    out, _ = run(x, Wqkv, bqkv, Wo, bo)
    return out
